# revision 21
# baseline (speedup 1.0000x reference)
"""Trainium2 Bass kernel for BlockChunkedActivityRoutedNet.

Reference computation (B=4096, IN_F=4096, 8 chunks of 512, top-2 by mean|x|,
chunk-expert Linears 512->512, concat -> final Linear 1024->4096):

    xr = x.reshape(B, 8, 512)
    activities = mean(|xr|, axis=(0, 2))            # over the WHOLE batch
    i0, i1 = top2(activities)                        # descending
    h = concat(xr[:, i0] @ Wc[i0] + bc[i0], xr[:, i1] @ Wc[i1] + bc[i1])
    out = h @ W_final + b_final

Distribution: data-parallel over the batch across 8 NeuronCores (512 rows
each). Cross-core activity exchange is done with 8 XOR-relative
remote_dma_broadcast rounds (peer SBUF writes + semaphores) instead of a
runtime AllReduce collective: the collective's barrier + mesh machinery cost
40-70us on the critical path, while the remote-DMA exchange costs a few us.
A compile-time prelude AllGather (bir_kernel_barrier_wait) guarantees every
core has entered the kernel (and cleared its semaphores in the preamble)
before any peer's remote write can land.

Host-side prep inside kernel(): casts to bf16 and packs each core's x shard
and the chunk weights as [1024, 2048] tables

    xg[c*128 + p, kt*512 + b] = x.T[c*512 + kt*128 + p, b]
    wg[c*128 + p, kt*512 + d] = W_chunks[c, kt*128 + p, d]

so that (a) chunk c's activity tile is the static row-slice [c*128:(c+1)*128]
with 4KB contiguous lines per partition, and (b) the post-routing gather of a
selected chunk is ONE indirect row-gather [128, 2048] (row = sel*128 + p)
with 4KB lines, instead of 8 1KB-row gathers.

Per core:
  activities: 8 chunk tiles [128, 2048] bf16, |.|-reduce over free dim
              (DVE half / ScalarE Abs+accum half) -> actcol [128, 8] f32.
  exchange:   8 remote_dma_broadcast rounds; round r sends actcol to core
              (me XOR r)'s recv[:, r*8:(r+1)*8]. After recv_sem==16, a
              pairwise tree sum over slots + ones-matmul partition reduce
              gives the global [1, 8] activity sums on every core.
  routing:    top2 via vector.max/max_index; indices broadcast to 128
              partitions via K=1 matmul; per-partition row offsets by iota
              math; chunk biases selected on-chip (PE-transpose + one-hot).
  gather:     2+2 indirect row-gathers (x and W tables, one per selected
              chunk).
  L1:         hT[s][d] = matmul over 4 k-tiles; bias added during PSUM
              eviction -> bf16.
  L2:         out = matmul over 8 k-tiles vs W_final ([128, 4096] bf16
              tiles); b_final added by DVE during PSUM eviction; bf16 out
              DMA (host upcasts to f32).
"""

import numpy as np
import ml_dtypes

import concourse.bass as bass
import concourse.bacc as bacc
import concourse.mybir as mybir
from concourse.tile import TileContext
from concourse.bass_utils import run_bass_kernel_spmd
from concourse.masks import make_identity

dt = mybir.dt
P = 128

NUM_CHUNKS = 8
TOP_K = 2
IN_F = 4096
HID_F = 4096
OUT_F = 4096
B = 4096
CIN = IN_F // NUM_CHUNKS      # 512
COUT = HID_F // NUM_CHUNKS    # 512
N_CORES = 8
BS = B // N_CORES             # 512 rows per core

BT = BS // P                  # 4 batch tiles per core
KT = CIN // P                 # 4 k-tiles per selected chunk
DT_ = COUT // P               # 4 d-tiles per selected chunk
KF = TOP_K * DT_              # 8 k-tiles for the final matmul
OT = OUT_F // 512             # 8 output column tiles of 512
GW = KT * BS                  # 2048 packed-table row width

_cache = {}


def _build():
    nc = bacc.Bacc(num_devices=N_CORES, name="chunk_routed_net",
                   num_swdge_queues=4)

    xg = nc.dram_tensor("xg_shard", [NUM_CHUNKS * P, GW], dt.bfloat16,
                        kind="ExternalInput")
    wg = nc.dram_tensor("wg_chunks", [NUM_CHUNKS * P, GW], dt.bfloat16,
                        kind="ExternalInput")
    bc_t = nc.dram_tensor("b_chunks", [NUM_CHUNKS, COUT], dt.float32,
                          kind="ExternalInput")
    Wf = nc.dram_tensor("W_final", [COUT * TOP_K, OUT_F], dt.bfloat16,
                        kind="ExternalInput")
    bf = nc.dram_tensor("b_final", [1, OUT_F], dt.float32, kind="ExternalInput")
    cmix_d = nc.dram_tensor("cmix", [P, 2 + NUM_CHUNKS], dt.float32,
                            kind="ExternalInput")
    out = nc.dram_tensor("out_shard", [BS, OUT_F], dt.bfloat16,
                         kind="ExternalOutput")

    # cross-core exchange semaphores (cleared by the per-exec preamble; the
    # entry AllGather below orders every peer's broadcast after every core's
    # preamble)
    recv_sem = nc.alloc_semaphore("act_recv_sem")
    send_sem = nc.alloc_semaphore("act_send_sem")

    # background collective: never consumed — its only purpose is to make
    # the NEFF cc-enabled so NRT initializes the D2D fabric at load (remote
    # DMA deliveries are ~1000x slower without it). Runs concurrently with
    # the real work; nothing waits on it.
    ag_in = nc.dram_tensor("bg_ag_in", [1, 1], dt.float32)
    ag_out = nc.dram_tensor("bg_ag_out", [N_CORES, 1], dt.float32)

    with TileContext(nc) as tc:
        with tc.tile_pool(name="consts", bufs=1) as consts, \
             tc.tile_pool(name="route", bufs=1) as route, \
             tc.tile_pool(name="xl", bufs=1) as xl_pool, \
             tc.tile_pool(name="gath", bufs=1) as gath, \
             tc.tile_pool(name="hts", bufs=1) as hts, \
             tc.tile_pool(name="bfinp", bufs=1) as bfinp, \
             tc.tile_pool(name="wfs", bufs=8) as wfs, \
             tc.tile_pool(name="outs", bufs=4) as outs:

            # ---------------- constants ----------------
            ones_col = consts.tile([P, 1], dt.float32)     # partition reduce
            nc.vector.memset(ones_col[:], 1.0)
            ones_k1 = consts.tile([1, P], dt.float32)      # K=1 bcast matmul
            nc.vector.memset(ones_k1[:], 1.0)
            ones_k1h = consts.tile([1, P], dt.bfloat16)    # K=1 bf16 bcast
            nc.vector.memset(ones_k1h[:], 1.0)
            ident = consts.tile([P, P], dt.float32)
            make_identity(nc, ident)
            # C_Rf[p, :] = p  /  C8f[p, c] = c  (host-provided; keeps gpsimd
            # off the iota ucode library so no mid-kernel library swap before
            # the remote-DMA desc-gens)
            cmix = consts.tile([P, 2 + NUM_CHUNKS], dt.float32)
            nc.scalar.dma_start(cmix[:], cmix_d[:])
            C_Rf = cmix[:, 0:2]
            C8f = cmix[:, 2:2 + NUM_CHUNKS]

            with tc.tile_pool(name="ps_early", bufs=1, space="PSUM") as ps_early:
                # ------------ activities from packed x (4KB lines) ---------
                actcol = route.tile([P, NUM_CHUNKS], dt.float32)
                scr = route.tile([P, GW], dt.bfloat16)  # ACT throwaway
                xls = []
                H = GW // 2
                for c in range(NUM_CHUNKS):
                    xlt = xl_pool.tile([P, GW], dt.bfloat16, tag=f"xl{c}",
                                       name=f"xl{c}")
                    nc.sync.dma_start(xlt[:, 0:H],
                                      xg[c * P:(c + 1) * P, 0:H])
                    nc.sync.dma_start(xlt[:, H:GW],
                                      xg[c * P:(c + 1) * P, H:GW])
                    xls.append(xlt)
                for c in range(NUM_CHUNKS):
                    if c % 2 == 0:
                        nc.vector.tensor_reduce(
                            actcol[:, c:c + 1], xls[c][:],
                            axis=mybir.AxisListType.X, op=mybir.AluOpType.add,
                            apply_absolute_value=True)
                    else:
                        nc.scalar.activation(
                            scr[:], xls[c][:],
                            mybir.ActivationFunctionType.Abs,
                            accum_out=actcol[:, c:c + 1])

                # ------------ cross-core exchange (remote DMA) ------------
                # round r sends my actcol to core (me XOR r)'s slot r; the
                # self-round r=0 is skipped (own partial summed from actcol).
                # Each arriving round bumps recv_sem by 2 -> 14 total.
                recv = route.tile([P, N_CORES * NUM_CHUNKS], dt.float32,
                                  name="act_recv")
                for r in range(1, N_CORES):
                    rdests = [None] * N_CORES
                    rdests[r] = (0, r)
                    nc.gpsimd.remote_dma_broadcast(
                        recv[:, r * NUM_CHUNKS:(r + 1) * NUM_CHUNKS],
                        actcol[:],
                        remote_sem=recv_sem, local_sem=send_sem,
                        rdests=rdests)
                nc.gpsimd.trigger_dma(count=None)

                # ---- work that fills the exchange wait ----
                # W_final prefetch: 8 x [128, 4096] bf16 (8KB lines), scalar q
                wf_t = []
                for kf in range(KF):
                    w = wfs.tile([P, OUT_F], dt.bfloat16, tag="wf",
                                 name=f"wf{kf}")
                    nc.scalar.dma_start(w[:], Wf[kf * P:(kf + 1) * P, :])
                    wf_t.append(w)
                # b_final broadcast [128, 4096]
                bfin = bfinp.tile([1, OUT_F], dt.float32)
                nc.scalar.dma_start(bfin[:], bf[:])
                bfin_h = bfinp.tile([1, OUT_F], dt.bfloat16)
                nc.vector.tensor_copy(bfin_h[:], bfin[:])
                bfin_bc = bfinp.tile([P, OUT_F], dt.float32)
                for o in range(OT):
                    sl = slice(o * 512, (o + 1) * 512)
                    ps_b = ps_early.tile([P, 512], dt.float32, tag="psb")
                    nc.tensor.matmul(ps_b[:], ones_k1h[:], bfin_h[:, sl],
                                     start=True, stop=True)
                    nc.vector.tensor_copy(bfin_bc[:, sl], ps_b[:])
                # b_chunks transpose
                b_sb = route.tile([NUM_CHUNKS, COUT], dt.float32)
                nc.scalar.dma_start(b_sb[:], bc_t[:])
                bT = route.tile([P, DT_ * NUM_CHUNKS], dt.float32)
                for d in range(DT_):
                    ps_t = ps_early.tile([P, NUM_CHUNKS], dt.float32, tag="pst")
                    nc.tensor.transpose(
                        ps_t[:], b_sb[:, d * P:(d + 1) * P],
                        ident[0:NUM_CHUNKS, 0:NUM_CHUNKS])
                    nc.scalar.copy(bT[:, d * NUM_CHUNKS:(d + 1) * NUM_CHUNKS],
                                   ps_t[:])

                # ------------ global activity sums ------------
                # acc = actcol + slot1 + ... + slot7. The first add carries a
                # hardware-only recv_sem>=14 wait injected post-scheduling
                # (the Tile sim can't model remote sem delivery).
                acts8 = route.tile([P, NUM_CHUNKS], dt.float32)
                first_add = nc.vector.tensor_tensor(
                    out=acts8[:], in0=actcol[:],
                    in1=recv[:, NUM_CHUNKS:2 * NUM_CHUNKS],
                    op=mybir.AluOpType.add)
                nc._act_recv_wait_fixup = (first_add.ins, recv_sem)
                for r in range(2, N_CORES):
                    nc.vector.tensor_tensor(
                        out=acts8[:], in0=acts8[:],
                        in1=recv[:, r * NUM_CHUNKS:(r + 1) * NUM_CHUNKS],
                        op=mybir.AluOpType.add)
                act_ps = ps_early.tile([1, NUM_CHUNKS], dt.float32, tag="psa")
                nc.tensor.matmul(act_ps[:], ones_col[:], acts8[:],
                                 start=True, stop=True)
                act_g = route.tile([1, NUM_CHUNKS], dt.float32)
                nc.scalar.copy(act_g[:], act_ps[:])

                # ------------ top-2 ------------
                maxv = route.tile([1, NUM_CHUNKS], dt.float32)
                maxi = route.tile([1, NUM_CHUNKS], dt.uint32)
                nc.vector.max(maxv[:], act_g[:])
                nc.vector.max_index(maxi[:], maxv[:], act_g[:])
                maxi_f = route.tile([1, NUM_CHUNKS], dt.float32)
                nc.vector.tensor_copy(maxi_f[:], maxi[:])

                # bcast[p, j] = idx[j] on every partition (K=1 matmul)
                bc_ps = ps_early.tile([P, NUM_CHUNKS], dt.float32, tag="psc")
                nc.tensor.matmul(bc_ps[:], ones_k1[:], maxi_f[:],
                                 start=True, stop=True)
                bcast = route.tile([P, NUM_CHUNKS], dt.float32)
                nc.vector.tensor_copy(bcast[:], bc_ps[:])

            # gather offsets: offR[p, s] = sel_s*128 + p
            bc128 = route.tile([P, TOP_K], dt.float32)
            nc.vector.tensor_scalar_mul(bc128[:], bcast[:, 0:TOP_K], 128.0)
            offR_f = route.tile([P, TOP_K], dt.float32)
            nc.vector.tensor_tensor(
                out=offR_f[:], in0=C_Rf[:, 0:TOP_K], in1=bc128[:],
                op=mybir.AluOpType.add)
            offR = route.tile([P, TOP_K], dt.int32)
            nc.vector.tensor_copy(offR[:], offR_f[:])

            # ------------ gathers: one row-gather per (tensor, slot) -------
            xgt = [gath.tile([P, GW], dt.bfloat16, tag=f"xg{s}", name=f"xg{s}")
                   for s in range(TOP_K)]
            wgt = [gath.tile([P, GW], dt.bfloat16, tag=f"wg{s}", name=f"wg{s}")
                   for s in range(TOP_K)]
            for s in range(TOP_K):
                nc.gpsimd.indirect_dma_start(
                    out=xgt[s][:], out_offset=None,
                    in_=xg[:],
                    in_offset=bass.IndirectOffsetOnAxis(
                        ap=offR[:, s:s + 1], axis=0))
                nc.gpsimd.indirect_dma_start(
                    out=wgt[s][:], out_offset=None,
                    in_=wg[:],
                    in_offset=bass.IndirectOffsetOnAxis(
                        ap=offR[:, s:s + 1], axis=0))

            # chunk-bias select: bias[s][d][p] = bT[p, d*8 + sel_s]
            onehot = route.tile([P, TOP_K * NUM_CHUNKS], dt.float32)
            for s in range(TOP_K):
                nc.vector.tensor_scalar(
                    onehot[:, s * NUM_CHUNKS:(s + 1) * NUM_CHUNKS], C8f[:],
                    bcast[:, s:s + 1], scalar2=None,
                    op0=mybir.AluOpType.is_equal)
            bsel = [[route.tile([P, 1], dt.float32, tag=f"bs{s}_{d}",
                                name=f"bs{s}_{d}")
                     for d in range(DT_)] for s in range(TOP_K)]
            btmp = route.tile([P, NUM_CHUNKS], dt.float32)
            for s in range(TOP_K):
                for d in range(DT_):
                    nc.vector.tensor_tensor(
                        out=btmp[:], in0=bT[:, d * NUM_CHUNKS:(d + 1) * NUM_CHUNKS],
                        in1=onehot[:, s * NUM_CHUNKS:(s + 1) * NUM_CHUNKS],
                        op=mybir.AluOpType.mult)
                    nc.vector.tensor_reduce(
                        bsel[s][d][:], btmp[:], axis=mybir.AxisListType.X,
                        op=mybir.AluOpType.add)

            with tc.tile_pool(name="ps_h", bufs=2, space="PSUM") as ps_h, \
                 tc.tile_pool(name="ps_o", bufs=6, space="PSUM") as ps_o:
                # ------------ L1: hT[s][d] = (x_sel @ Wc[sel]).T + b -------
                hT = [[hts.tile([P, BS], dt.bfloat16, tag=f"ht{s}_{d}",
                                name=f"ht{s}_{d}")
                       for d in range(DT_)] for s in range(TOP_K)]

                def l1_chunk(s):
                    for d in range(DT_):
                        ph = ps_h.tile([P, BS], dt.float32, tag="ph",
                                       name=f"ph{s}_{d}")
                        for kt in range(KT):
                            base = kt * 512 + d * P
                            nc.tensor.matmul(
                                ph[:], wgt[s][:, base:base + P],
                                xgt[s][:, kt * 512:(kt + 1) * 512],
                                start=(kt == 0), stop=(kt == KT - 1))
                        nc.scalar.activation(
                            hT[s][d][:], ph[:],
                            mybir.ActivationFunctionType.Identity,
                            bias=bsel[s][d][:, 0:1])

                l1_chunk(0)

                # pre-start six psum groups (all of o=0 plus o=1 bt=0,1)
                # on slot-0 hT while slot-1 gathers/L1 are still in flight
                PRE = [(0, 0), (0, 1), (0, 2), (0, 3), (1, 0), (1, 1)]
                pre = {}
                for (po_, bt) in PRE:
                    po = ps_o.tile([P, 512], dt.float32, tag="po",
                                   name=f"po_pre{po_}_{bt}")
                    osl = slice(po_ * 512, (po_ + 1) * 512)
                    for kf in range(DT_):
                        nc.tensor.matmul(
                            po[:], hT[0][kf][:, bt * P:(bt + 1) * P],
                            wf_t[kf][:, osl],
                            start=(kf == 0), stop=False)
                    pre[(po_, bt)] = po

                l1_chunk(1)

                # ------------ L2: out = h @ W_final + b_final --------------
                for o in range(OT):
                    osl = slice(o * 512, (o + 1) * 512)
                    for bt in range(BT):
                        if (o, bt) in pre:
                            po = pre[(o, bt)]
                            kfs = range(DT_, KF)
                        else:
                            po = ps_o.tile([P, 512], dt.float32, tag="po",
                                           name=f"po{o}_{bt}")
                            kfs = range(KF)
                        for kf in kfs:
                            s, d = divmod(kf, DT_)
                            nc.tensor.matmul(
                                po[:], hT[s][d][:, bt * P:(bt + 1) * P],
                                wf_t[kf][:, osl],
                                start=(kf == 0), stop=(kf == KF - 1))
                        ot_sb = outs.tile([P, 512], dt.bfloat16, tag="ot",
                                          name=f"ot{o}_{bt}")
                        nc.vector.tensor_tensor(
                            out=ot_sb[:], in0=po[:], in1=bfin_bc[:, osl],
                            op=mybir.AluOpType.add)
                        nc.sync.dma_start(
                            out[bt * P:(bt + 1) * P, osl], ot_sb[:])

    # hardware-only gate: the slot-sum must not start until all 7 peer
    # broadcasts have landed (recv_sem >= 14). Injected after Tile
    # scheduling so the single-core scheduling sim (which cannot model
    # remote sem delivery) doesn't deadlock on it.
    import bass_rust
    ins, sem = nc._act_recv_wait_fixup
    si = ins.sync_info
    new_wait = bass_rust.SyncWait(
        sync_type="semaphore", id=sem.num, ant_name=sem.name,
        wait_mode="sem-ge-imm", wait_value=2 * (N_CORES - 1), wait_reg=None)
    ins.sync_info = bass_rust.SyncInfo(
        on_wait=list(si.on_wait) + [new_wait],
        on_update=list(si.on_update))
    assert "act_recv_sem" in str(ins.sync_info)

    # cc-enable the NEFF (D2D route programming at load) without paying for
    # any runtime collective instruction
    nc.has_collectives = True

    nc.compile()
    return nc


def _pack_table(a):
    # [8, 512, N] -> [1024, 4*N] with row (c*128+p) = a[c, kt*128+p, :] for
    # kt = 0..3 laid side by side
    n = a.shape[-1]
    return np.ascontiguousarray(
        a.reshape(NUM_CHUNKS, KT, P, n).transpose(0, 2, 1, 3)
        .reshape(NUM_CHUNKS * P, KT * n))


def kernel(x, W_chunks, b_chunks, W_final, b_final):
    bf16 = ml_dtypes.bfloat16
    x = np.asarray(x, dtype=np.float32).astype(bf16)
    W_chunks = np.asarray(W_chunks, dtype=np.float32).astype(bf16)
    W_final = np.asarray(W_final, dtype=np.float32).astype(bf16)
    b_chunks = np.ascontiguousarray(np.asarray(b_chunks, dtype=np.float32))
    b_final = np.ascontiguousarray(
        np.asarray(b_final, dtype=np.float32).reshape(1, OUT_F))

    wg = _pack_table(W_chunks)

    cmix = np.zeros((P, 2 + NUM_CHUNKS), dtype=np.float32)
    cmix[:, 0:2] = np.arange(P, dtype=np.float32)[:, None]
    cmix[:, 2:] = np.arange(NUM_CHUNKS, dtype=np.float32)[None, :]

    if "nc" not in _cache:
        _cache["nc"] = _build()
    nc = _cache["nc"]

    in_maps = []
    for c in range(N_CORES):
        shard = x[c * BS:(c + 1) * BS]              # [512, 4096]
        xt = shard.T.reshape(NUM_CHUNKS, CIN, BS)   # [8, 512, 512]
        in_maps.append({
            "xg_shard": _pack_table(xt),
            "wg_chunks": wg,
            "b_chunks": b_chunks,
            "W_final": W_final,
            "b_final": b_final,
            "cmix": cmix,
        })

    res = run_bass_kernel_spmd(nc, in_maps, core_ids=list(range(N_CORES)))
    kernel.last_result = res
    return np.concatenate(
        [res.results[c]["out_shard"].astype(np.float32)
         for c in range(N_CORES)], axis=0)


kernel.last_result = None


# revision 26
# speedup vs baseline: 107.5579x; 107.5579x over previous
"""Trainium2 Bass kernel for BlockChunkedActivityRoutedNet.

Reference computation (B=4096, IN_F=4096, 8 chunks of 512, top-2 by mean|x|,
chunk-expert Linears 512->512, concat -> final Linear 1024->4096):

    xr = x.reshape(B, 8, 512)
    activities = mean(|xr|, axis=(0, 2))            # over the WHOLE batch
    i0, i1 = top2(activities)                        # descending
    h = concat(xr[:, i0] @ Wc[i0] + bc[i0], xr[:, i1] @ Wc[i1] + bc[i1])
    out = h @ W_final + b_final

Distribution: data-parallel over the batch across 8 NeuronCores (512 rows
each). Cross-core activity exchange is done with 8 XOR-relative
remote_dma_broadcast rounds (peer SBUF writes + semaphores) instead of a
runtime AllReduce collective: the collective's barrier + mesh machinery cost
40-70us on the critical path, while the remote-DMA exchange costs a few us.
A compile-time prelude AllGather (bir_kernel_barrier_wait) guarantees every
core has entered the kernel (and cleared its semaphores in the preamble)
before any peer's remote write can land.

Host-side prep inside kernel(): casts to bf16 and packs each core's x shard
and the chunk weights as [1024, 2048] tables

    xg[c*128 + p, kt*512 + b] = x.T[c*512 + kt*128 + p, b]
    wg[c*128 + p, kt*512 + d] = W_chunks[c, kt*128 + p, d]

so that (a) chunk c's activity tile is the static row-slice [c*128:(c+1)*128]
with 4KB contiguous lines per partition, and (b) the post-routing gather of a
selected chunk is ONE indirect row-gather [128, 2048] (row = sel*128 + p)
with 4KB lines, instead of 8 1KB-row gathers.

Per core:
  activities: 8 chunk tiles [128, 2048] bf16, |.|-reduce over free dim
              (DVE half / ScalarE Abs+accum half) -> actcol [128, 8] f32.
  exchange:   8 remote_dma_broadcast rounds; round r sends actcol to core
              (me XOR r)'s recv[:, r*8:(r+1)*8]. After recv_sem==16, a
              pairwise tree sum over slots + ones-matmul partition reduce
              gives the global [1, 8] activity sums on every core.
  routing:    top2 via vector.max/max_index; indices broadcast to 128
              partitions via K=1 matmul; per-partition row offsets by iota
              math; chunk biases selected on-chip (PE-transpose + one-hot).
  gather:     2+2 indirect row-gathers (x and W tables, one per selected
              chunk).
  L1:         hT[s][d] = matmul over 4 k-tiles; bias added during PSUM
              eviction -> bf16.
  L2:         out = matmul over 8 k-tiles vs W_final ([128, 4096] bf16
              tiles); b_final added by DVE during PSUM eviction; bf16 out
              DMA (host upcasts to f32).
"""

import numpy as np
import ml_dtypes

import concourse.bass as bass
import concourse.bacc as bacc
import concourse.mybir as mybir
from concourse.tile import TileContext
from concourse.bass_utils import run_bass_kernel_spmd
from concourse.masks import make_identity

dt = mybir.dt
P = 128

NUM_CHUNKS = 8
TOP_K = 2
IN_F = 4096
HID_F = 4096
OUT_F = 4096
B = 4096
CIN = IN_F // NUM_CHUNKS      # 512
COUT = HID_F // NUM_CHUNKS    # 512
N_CORES = 8
BS = B // N_CORES             # 512 rows per core

BT = BS // P                  # 4 batch tiles per core
KT = CIN // P                 # 4 k-tiles per selected chunk
DT_ = COUT // P               # 4 d-tiles per selected chunk
KF = TOP_K * DT_              # 8 k-tiles for the final matmul
OT = OUT_F // 512             # 8 output column tiles of 512
GW = KT * BS                  # 2048 packed-table row width

_cache = {}


def _build():
    nc = bacc.Bacc(num_devices=N_CORES, name="chunk_routed_net",
                   num_swdge_queues=4)

    xg = nc.dram_tensor("xg_shard", [NUM_CHUNKS * P, GW], dt.bfloat16,
                        kind="ExternalInput")
    wg = nc.dram_tensor("wg_chunks", [NUM_CHUNKS * P, GW], dt.bfloat16,
                        kind="ExternalInput")
    bc_t = nc.dram_tensor("b_chunks", [NUM_CHUNKS, COUT], dt.float32,
                          kind="ExternalInput")
    Wf = nc.dram_tensor("W_final", [COUT * TOP_K, OUT_F], dt.bfloat16,
                        kind="ExternalInput")
    bf = nc.dram_tensor("b_final", [1, OUT_F], dt.float32, kind="ExternalInput")
    cmix_d = nc.dram_tensor("cmix", [P, 2 + NUM_CHUNKS], dt.float32,
                            kind="ExternalInput")
    out = nc.dram_tensor("out_shard", [BS, OUT_F], dt.bfloat16,
                         kind="ExternalOutput")

    # cross-core exchange semaphores (cleared by the per-exec preamble; the
    # entry AllGather below orders every peer's broadcast after every core's
    # preamble)
    # cross-core activity exchange: AllReduce of the [128, 8] per-partition
    # per-chunk |x| partials (partition reduce happens after, locally)
    cc_in = nc.dram_tensor("cc_in", [P, NUM_CHUNKS], dt.float32)
    cc_out = nc.dram_tensor("cc_out", [P, NUM_CHUNKS], dt.float32)

    with TileContext(nc) as tc:
        with tc.tile_pool(name="consts", bufs=1) as consts, \
             tc.tile_pool(name="route", bufs=1) as route, \
             tc.tile_pool(name="xl", bufs=1) as xl_pool, \
             tc.tile_pool(name="gath", bufs=1) as gath, \
             tc.tile_pool(name="hts", bufs=1) as hts, \
             tc.tile_pool(name="bfinp", bufs=1) as bfinp, \
             tc.tile_pool(name="wfs", bufs=8) as wfs, \
             tc.tile_pool(name="outs", bufs=4) as outs:

            # ---------------- constants ----------------
            ones_col = consts.tile([P, 1], dt.float32)     # partition reduce
            nc.vector.memset(ones_col[:], 1.0)
            ones_k1 = consts.tile([1, P], dt.float32)      # K=1 bcast matmul
            nc.vector.memset(ones_k1[:], 1.0)
            ones_k1h = consts.tile([1, P], dt.bfloat16)    # K=1 bf16 bcast
            nc.vector.memset(ones_k1h[:], 1.0)
            ident = consts.tile([P, P], dt.float32)
            make_identity(nc, ident)
            # C_Rf[p, :] = p  /  C8f[p, c] = c  (host-provided; keeps gpsimd
            # off the iota ucode library so no mid-kernel library swap before
            # the remote-DMA desc-gens)
            cmix = consts.tile([P, 2 + NUM_CHUNKS], dt.float32)
            nc.scalar.dma_start(cmix[:], cmix_d[:])
            C_Rf = cmix[:, 0:2]
            C8f = cmix[:, 2:2 + NUM_CHUNKS]

            with tc.tile_pool(name="ps_early", bufs=1, space="PSUM") as ps_early:
                # ------------ activities from packed x (4KB lines) ---------
                actcol = route.tile([P, NUM_CHUNKS], dt.float32)
                scr = route.tile([P, GW], dt.bfloat16)  # ACT throwaway
                xls = []
                H = GW // 2
                for c in range(NUM_CHUNKS):
                    xlt = xl_pool.tile([P, GW], dt.bfloat16, tag=f"xl{c}",
                                       name=f"xl{c}")
                    nc.sync.dma_start(xlt[:, 0:H],
                                      xg[c * P:(c + 1) * P, 0:H])
                    nc.scalar.dma_start(xlt[:, H:GW],
                                        xg[c * P:(c + 1) * P, H:GW])
                    xls.append(xlt)
                for c in range(NUM_CHUNKS):
                    if c % 2 == 0:
                        nc.vector.tensor_reduce(
                            actcol[:, c:c + 1], xls[c][:],
                            axis=mybir.AxisListType.X, op=mybir.AluOpType.add,
                            apply_absolute_value=True)
                    else:
                        nc.scalar.activation(
                            scr[:], xls[c][:],
                            mybir.ActivationFunctionType.Abs,
                            accum_out=actcol[:, c:c + 1])

                # ------------ cross-core exchange (AllReduce) ------------
                nc.gpsimd.dma_start(cc_in.ap(), actcol[:])
                nc.gpsimd.collective_compute(
                    "AllReduce", mybir.AluOpType.add,
                    replica_groups=[list(range(N_CORES))],
                    ins=[cc_in.ap()], outs=[cc_out.ap()])

                # ---- work that fills the exchange wait ----
                # W_final prefetch: 8 x [128, 4096] bf16 (8KB lines), scalar q
                wf_t = []
                for kf in range(KF):
                    w = wfs.tile([P, OUT_F], dt.bfloat16, tag="wf",
                                 name=f"wf{kf}")
                    nc.scalar.dma_start(w[:], Wf[kf * P:(kf + 1) * P, :])
                    wf_t.append(w)
                # b_final broadcast [128, 4096]
                bfin = bfinp.tile([1, OUT_F], dt.float32)
                nc.scalar.dma_start(bfin[:], bf[:])
                bfin_h = bfinp.tile([1, OUT_F], dt.bfloat16)
                nc.vector.tensor_copy(bfin_h[:], bfin[:])
                bfin_bc = bfinp.tile([P, OUT_F], dt.float32)
                for o in range(OT):
                    sl = slice(o * 512, (o + 1) * 512)
                    ps_b = ps_early.tile([P, 512], dt.float32, tag="psb")
                    nc.tensor.matmul(ps_b[:], ones_k1h[:], bfin_h[:, sl],
                                     start=True, stop=True)
                    nc.vector.tensor_copy(bfin_bc[:, sl], ps_b[:])
                # b_chunks transpose
                b_sb = route.tile([NUM_CHUNKS, COUT], dt.float32)
                nc.scalar.dma_start(b_sb[:], bc_t[:])
                bT = route.tile([P, DT_ * NUM_CHUNKS], dt.float32)
                for d in range(DT_):
                    ps_t = ps_early.tile([P, NUM_CHUNKS], dt.float32, tag="pst")
                    nc.tensor.transpose(
                        ps_t[:], b_sb[:, d * P:(d + 1) * P],
                        ident[0:NUM_CHUNKS, 0:NUM_CHUNKS])
                    nc.scalar.copy(bT[:, d * NUM_CHUNKS:(d + 1) * NUM_CHUNKS],
                                   ps_t[:])

                # ------------ global activity sums ------------
                acts8 = route.tile([P, NUM_CHUNKS], dt.float32)
                nc.gpsimd.dma_start(acts8[:], cc_out.ap())
                act_ps = ps_early.tile([1, NUM_CHUNKS], dt.float32, tag="psa")
                nc.tensor.matmul(act_ps[:], ones_col[:], acts8[:],
                                 start=True, stop=True)
                act_g = route.tile([1, NUM_CHUNKS], dt.float32)
                nc.scalar.copy(act_g[:], act_ps[:])

                # ------------ top-2 ------------
                maxv = route.tile([1, NUM_CHUNKS], dt.float32)
                maxi = route.tile([1, NUM_CHUNKS], dt.uint32)
                nc.vector.max(maxv[:], act_g[:])
                nc.vector.max_index(maxi[:], maxv[:], act_g[:])
                maxi_f = route.tile([1, NUM_CHUNKS], dt.float32)
                nc.vector.tensor_copy(maxi_f[:], maxi[:])

                # bcast[p, j] = idx[j] on every partition (K=1 matmul)
                bc_ps = ps_early.tile([P, NUM_CHUNKS], dt.float32, tag="psc")
                nc.tensor.matmul(bc_ps[:], ones_k1[:], maxi_f[:],
                                 start=True, stop=True)
                bcast = route.tile([P, NUM_CHUNKS], dt.float32)
                nc.vector.tensor_copy(bcast[:], bc_ps[:])

            # gather offsets: offR[p, s] = sel_s*128 + p
            bc128 = route.tile([P, TOP_K], dt.float32)
            nc.vector.tensor_scalar_mul(bc128[:], bcast[:, 0:TOP_K], 128.0)
            offR_f = route.tile([P, TOP_K], dt.float32)
            nc.vector.tensor_tensor(
                out=offR_f[:], in0=C_Rf[:, 0:TOP_K], in1=bc128[:],
                op=mybir.AluOpType.add)
            offR = route.tile([P, TOP_K], dt.int32)
            nc.vector.tensor_copy(offR[:], offR_f[:])

            # ------------ gathers: one row-gather per (tensor, slot) -------
            xgt = [gath.tile([P, GW], dt.bfloat16, tag=f"xg{s}", name=f"xg{s}")
                   for s in range(TOP_K)]
            wgt = [gath.tile([P, GW], dt.bfloat16, tag=f"wg{s}", name=f"wg{s}")
                   for s in range(TOP_K)]
            for s in range(TOP_K):
                nc.gpsimd.indirect_dma_start(
                    out=xgt[s][:], out_offset=None,
                    in_=xg[:],
                    in_offset=bass.IndirectOffsetOnAxis(
                        ap=offR[:, s:s + 1], axis=0))
                nc.gpsimd.indirect_dma_start(
                    out=wgt[s][:], out_offset=None,
                    in_=wg[:],
                    in_offset=bass.IndirectOffsetOnAxis(
                        ap=offR[:, s:s + 1], axis=0))

            # chunk-bias select: bias[s][d][p] = bT[p, d*8 + sel_s]
            onehot = route.tile([P, TOP_K * NUM_CHUNKS], dt.float32)
            for s in range(TOP_K):
                nc.vector.tensor_scalar(
                    onehot[:, s * NUM_CHUNKS:(s + 1) * NUM_CHUNKS], C8f[:],
                    bcast[:, s:s + 1], scalar2=None,
                    op0=mybir.AluOpType.is_equal)
            bsel = [[route.tile([P, 1], dt.float32, tag=f"bs{s}_{d}",
                                name=f"bs{s}_{d}")
                     for d in range(DT_)] for s in range(TOP_K)]
            btmp = route.tile([P, NUM_CHUNKS], dt.float32)
            for s in range(TOP_K):
                for d in range(DT_):
                    nc.vector.tensor_tensor(
                        out=btmp[:], in0=bT[:, d * NUM_CHUNKS:(d + 1) * NUM_CHUNKS],
                        in1=onehot[:, s * NUM_CHUNKS:(s + 1) * NUM_CHUNKS],
                        op=mybir.AluOpType.mult)
                    nc.vector.tensor_reduce(
                        bsel[s][d][:], btmp[:], axis=mybir.AxisListType.X,
                        op=mybir.AluOpType.add)

            with tc.tile_pool(name="ps_h", bufs=2, space="PSUM") as ps_h, \
                 tc.tile_pool(name="ps_o", bufs=6, space="PSUM") as ps_o:
                # ------------ L1: hT[s][d] = (x_sel @ Wc[sel]).T + b -------
                hT = [[hts.tile([P, BS], dt.bfloat16, tag=f"ht{s}_{d}",
                                name=f"ht{s}_{d}")
                       for d in range(DT_)] for s in range(TOP_K)]

                def l1_chunk(s):
                    for d in range(DT_):
                        ph = ps_h.tile([P, BS], dt.float32, tag="ph",
                                       name=f"ph{s}_{d}")
                        for kt in range(KT):
                            base = kt * 512 + d * P
                            nc.tensor.matmul(
                                ph[:], wgt[s][:, base:base + P],
                                xgt[s][:, kt * 512:(kt + 1) * 512],
                                start=(kt == 0), stop=(kt == KT - 1))
                        nc.scalar.activation(
                            hT[s][d][:], ph[:],
                            mybir.ActivationFunctionType.Identity,
                            bias=bsel[s][d][:, 0:1])

                l1_chunk(0)

                # pre-start six psum groups (all of o=0 plus o=1 bt=0,1)
                # on slot-0 hT while slot-1 gathers/L1 are still in flight
                PRE = [(0, 0), (0, 1), (0, 2), (0, 3), (1, 0), (1, 1)]
                pre = {}
                for (po_, bt) in PRE:
                    po = ps_o.tile([P, 512], dt.float32, tag="po",
                                   name=f"po_pre{po_}_{bt}")
                    osl = slice(po_ * 512, (po_ + 1) * 512)
                    for kf in range(DT_):
                        nc.tensor.matmul(
                            po[:], hT[0][kf][:, bt * P:(bt + 1) * P],
                            wf_t[kf][:, osl],
                            start=(kf == 0), stop=False)
                    pre[(po_, bt)] = po

                l1_chunk(1)

                # ------------ L2: out = h @ W_final + b_final --------------
                for o in range(OT):
                    osl = slice(o * 512, (o + 1) * 512)
                    for bt in range(BT):
                        if (o, bt) in pre:
                            po = pre[(o, bt)]
                            kfs = range(DT_, KF)
                        else:
                            po = ps_o.tile([P, 512], dt.float32, tag="po",
                                           name=f"po{o}_{bt}")
                            kfs = range(KF)
                        for kf in kfs:
                            s, d = divmod(kf, DT_)
                            nc.tensor.matmul(
                                po[:], hT[s][d][:, bt * P:(bt + 1) * P],
                                wf_t[kf][:, osl],
                                start=(kf == 0), stop=(kf == KF - 1))
                        ot_sb = outs.tile([P, 512], dt.bfloat16, tag="ot",
                                          name=f"ot{o}_{bt}")
                        nc.vector.tensor_tensor(
                            out=ot_sb[:], in0=po[:], in1=bfin_bc[:, osl],
                            op=mybir.AluOpType.add)
                        nc.sync.dma_start(
                            out[bt * P:(bt + 1) * P, osl], ot_sb[:])

    nc.compile()
    return nc


def _pack_table(a):
    # [8, 512, N] -> [1024, 4*N] with row (c*128+p) = a[c, kt*128+p, :] for
    # kt = 0..3 laid side by side
    n = a.shape[-1]
    return np.ascontiguousarray(
        a.reshape(NUM_CHUNKS, KT, P, n).transpose(0, 2, 1, 3)
        .reshape(NUM_CHUNKS * P, KT * n))


def kernel(x, W_chunks, b_chunks, W_final, b_final):
    bf16 = ml_dtypes.bfloat16
    x = np.asarray(x, dtype=np.float32).astype(bf16)
    W_chunks = np.asarray(W_chunks, dtype=np.float32).astype(bf16)
    W_final = np.asarray(W_final, dtype=np.float32).astype(bf16)
    b_chunks = np.ascontiguousarray(np.asarray(b_chunks, dtype=np.float32))
    b_final = np.ascontiguousarray(
        np.asarray(b_final, dtype=np.float32).reshape(1, OUT_F))

    wg = _pack_table(W_chunks)

    cmix = np.zeros((P, 2 + NUM_CHUNKS), dtype=np.float32)
    cmix[:, 0:2] = np.arange(P, dtype=np.float32)[:, None]
    cmix[:, 2:] = np.arange(NUM_CHUNKS, dtype=np.float32)[None, :]

    if "nc" not in _cache:
        _cache["nc"] = _build()
    nc = _cache["nc"]

    in_maps = []
    for c in range(N_CORES):
        shard = x[c * BS:(c + 1) * BS]              # [512, 4096]
        xt = shard.T.reshape(NUM_CHUNKS, CIN, BS)   # [8, 512, 512]
        in_maps.append({
            "xg_shard": _pack_table(xt),
            "wg_chunks": wg,
            "b_chunks": b_chunks,
            "W_final": W_final,
            "b_final": b_final,
            "cmix": cmix,
        })

    res = run_bass_kernel_spmd(nc, in_maps, core_ids=list(range(N_CORES)))
    kernel.last_result = res
    return np.concatenate(
        [res.results[c]["out_shard"].astype(np.float32)
         for c in range(N_CORES)], axis=0)


kernel.last_result = None
